# revision 5
# baseline (speedup 1.0000x reference)
"""MoE layer (B=4, N=2048, C=1024, F=4096, E=8, top-2) on 8 trn2 NeuronCores.

Sharding: expert-parallel. The host computes the (tiny, ~0.1% of FLOPs)
router and dispatches each expert's tokens to one core as part of sharding;
each core runs its expert's full FFN  relu(xg @ w1.T + b1) @ w2.T + b2,
gated by the combine weight, over its gathered tokens.  The host combine
scatter-adds the per-expert results back into the full output.

Device kernel (per core, SPMD, identical program):
  inputs : xgT [1024, cap]  (gathered tokens, transposed)
           w1t [1024, 4096] (w1[e].T)   w2t [4096, 1024] (w2[e].T)
           b1r [128, 32]    (b1[e] partition-major)
           b2r [128, 1024]  (b2[e] replicated over partitions)
           wg  [128, cap/128] (combine weights, partition-major)
  output : yg  [cap, 1024]
Matmuls run as float32r (FP22 multiply, fp32 accumulate) — 4x the fp32 rate.
"""

import numpy as np

P = 128
C = 1024
F = 4096
E = 8
SCH = 384  # token chunk: 3 PSUM banks (x 2 C-halves) for y + 2 for h = 8


def _build(cap: int):
    import concourse.mybir as mybir
    from concourse import bacc
    from concourse.tile import TileContext

    f32 = mybir.dt.float32
    f32r = mybir.dt.float32r
    nS = cap // SCH
    nc = bacc.Bacc(None, target_bir_lowering=False)

    xgT = nc.dram_tensor("xgT", [C, cap], f32, kind="ExternalInput")
    w1t = nc.dram_tensor("w1t", [C, F], f32, kind="ExternalInput")
    w2t = nc.dram_tensor("w2t", [F, C], f32, kind="ExternalInput")
    b1r = nc.dram_tensor("b1r", [P, F // P], f32, kind="ExternalInput")
    b2r = nc.dram_tensor("b2r", [P, C], f32, kind="ExternalInput")
    wg = nc.dram_tensor("wg", [P, cap // P], f32, kind="ExternalInput")
    yg = nc.dram_tensor("yg", [cap, C], f32, kind="ExternalOutput")

    w1v = w1t.ap().rearrange("(co ci) f -> ci co f", ci=P)  # [128, 8, F]
    xgv = xgT.ap().rearrange("(co ci) n -> ci co n", ci=P)  # [128, 8, cap]

    with TileContext(nc) as tc:
        with (
            tc.tile_pool(name="consts", bufs=1) as consts,
            tc.tile_pool(name="wpool", bufs=4) as wpool,
            tc.tile_pool(name="xpool", bufs=2) as xpool,
            tc.tile_pool(name="hpool", bufs=3) as hpool,
            tc.tile_pool(name="ypool", bufs=3) as ypool,
            tc.tile_pool(name="psum_h", bufs=2, space="PSUM") as psum_h,
            tc.tile_pool(name="psum_y", bufs=1, space="PSUM") as psum_y,
        ):
            b1_sb = consts.tile([P, F // P], f32)
            nc.sync.dma_start(b1_sb[:], b1r[:, :])
            b2_sb = consts.tile([P, C], f32)
            nc.sync.dma_start(b2_sb[:], b2r[:, :])
            wg_sb = consts.tile([P, cap // P], f32)
            nc.sync.dma_start(wg_sb[:], wg[:, :])

            for s in range(nS):
                xg_s = xpool.tile([P, 8, SCH], f32r, tag="xg")
                nc.sync.dma_start(xg_s[:], xgv[:, :, s * SCH : (s + 1) * SCH].bitcast(f32r))

                yps = [
                    [
                        psum_y.tile(
                            [P, 512], f32, tag=f"y_{t}_{cc}", name=f"y_{t}_{cc}"
                        )
                        for cc in range(2)
                    ]
                    for t in range(3)
                ]

                for f in range(F // P):  # 32
                    w1c = wpool.tile([P, 8, P], f32r, tag="w1c")
                    nc.sync.dma_start(w1c[:], w1v[:, :, f * P : (f + 1) * P].bitcast(f32r))
                    w2c = wpool.tile([P, C], f32r, tag="w2c")
                    nc.sync.dma_start(w2c[:], w2t[f * P : (f + 1) * P, :].bitcast(f32r))

                    hps = psum_h.tile([P, SCH], f32, tag="h")
                    for c in range(8):
                        nc.tensor.matmul(
                            hps[:],
                            lhsT=w1c[:, c, :],
                            rhs=xg_s[:, c, :],
                            start=(c == 0),
                            stop=(c == 7),
                        )
                    hT = hpool.tile([P, SCH], f32r, tag="hT")
                    nc.scalar.activation(
                        hT[:],
                        hps[:],
                        mybir.ActivationFunctionType.Relu,
                        bias=b1_sb[:, f : f + 1],
                        scale=1.0,
                    )
                    for t in range(3):
                        for cc in range(2):
                            nc.tensor.matmul(
                                yps[t][cc][:],
                                lhsT=hT[:, t * P : (t + 1) * P],
                                rhs=w2c[:, cc * 512 : (cc + 1) * 512],
                                start=(f == 0),
                                stop=(f == F // P - 1),
                            )

                for t in range(3):
                    y_sb = ypool.tile([P, C], f32, tag="y_sb")
                    for cc in range(2):
                        sl = slice(cc * 512, (cc + 1) * 512)
                        nc.vector.tensor_add(y_sb[:, sl], yps[t][cc][:], b2_sb[:, sl])
                    yf = ypool.tile([P, C], f32, tag="yf")
                    nc.scalar.mul(yf[:], y_sb[:], wg_sb[:, s * 3 + t : s * 3 + t + 1])
                    nc.sync.dma_start(
                        yg[(s * 3 + t) * P : (s * 3 + t + 1) * P, :], yf[:]
                    )
    nc.compile()
    return nc


_CACHE = {}
_TRACE = False  # test harness sets True to capture an NTFF profile
_LAST_RES = None


def _get_nc(cap):
    if cap not in _CACHE:
        _CACHE[cap] = _build(cap)
    return _CACHE[cap]


def _route(x_flat, router_w):
    """Top-2 routing, float64 for stable selection. Returns idx/weights per expert."""
    logits = x_flat.astype(np.float64) @ router_w.astype(np.float64).T
    t = np.exp(logits - logits.max(-1, keepdims=True))
    p = t / t.sum(-1, keepdims=True)
    top2 = np.argsort(-p, axis=-1)[:, :2]
    pv = np.take_along_axis(p, top2, axis=-1)
    wn = pv / (pv.sum(-1, keepdims=True) + 1e-9)
    return top2, wn


def kernel(x, router_w, w1, b1, w2, b2):
    from concourse.bass_utils import run_bass_kernel_spmd

    Bx, Nx, Cx = x.shape
    x_flat = np.ascontiguousarray(x.reshape(-1, Cx))
    T = x_flat.shape[0]

    top2, wn = _route(x_flat, router_w)
    idxs, gates = [], []
    for e in range(E):
        sel = top2 == e
        we = np.where(sel, wn, 0.0).sum(-1)
        idx = np.nonzero(sel.any(-1))[0]
        idxs.append(idx)
        gates.append(we[idx].astype(np.float32))
    cap = max(len(i) for i in idxs)
    cap = ((cap + SCH - 1) // SCH) * SCH

    nc = _get_nc(cap)

    in_maps = []
    for e in range(E):
        n_e = len(idxs[e])
        xg = np.zeros((cap, Cx), np.float32)
        xg[:n_e] = x_flat[idxs[e]]
        wg = np.zeros(cap, np.float32)
        wg[:n_e] = gates[e]
        in_maps.append(
            {
                "xgT": np.ascontiguousarray(xg.T),
                "w1t": np.ascontiguousarray(w1[e].T),
                "w2t": np.ascontiguousarray(w2[e].T),
                "b1r": np.ascontiguousarray(b1[e].reshape(F // P, P).T),
                "b2r": np.ascontiguousarray(np.broadcast_to(b2[e], (P, Cx))),
                "wg": np.ascontiguousarray(wg.reshape(cap // P, P).T),
            }
        )

    global _LAST_RES
    res = run_bass_kernel_spmd(nc, in_maps, core_ids=list(range(E)), trace=_TRACE)
    _LAST_RES = res

    out = np.zeros((T, Cx), np.float32)
    for e in range(E):
        n_e = len(idxs[e])
        out[idxs[e]] += res.results[e]["yg"][:n_e]
    return out.reshape(Bx, Nx, Cx)


# revision 8
# speedup vs baseline: 1.3047x; 1.3047x over previous
"""MoE layer (B=4, N=2048, C=1024, F=4096, E=8, top-2) on 8 trn2 NeuronCores.

Sharding: expert-parallel. The host computes the (tiny, ~0.1% of FLOPs)
router and dispatches each expert's tokens to one core as part of sharding;
each core runs its expert's full FFN  relu(xg @ w1.T + b1) @ w2.T + b2,
gated by the combine weight, over its gathered tokens.  The host combine
scatter-adds the per-expert results back into the full output.

Device kernel (per core, SPMD, identical program):
  inputs : xgT [1024, cap]  (gathered tokens, transposed)
           w1t [1024, 4096] (w1[e].T)   w2t [4096, 1024] (w2[e].T)
           b1r [128, 32]    (b1[e] partition-major)
           b2r [128, 1024]  (b2[e] replicated over partitions)
           wg  [128, cap/128] (combine weights, partition-major)
  output : yg  [cap, 1024]
Matmuls run as float32r (FP22 multiply, fp32 accumulate) — 4x the fp32 rate.
"""

import numpy as np

P = 128
C = 1024
F = 4096
E = 8
SCH = 384  # token chunk: 3 PSUM banks (x 2 C-halves) for y + 2 for h = 8


def _build(cap: int):
    import concourse.mybir as mybir
    from concourse import bacc
    from concourse.tile import TileContext

    f32 = mybir.dt.float32
    f32r = mybir.dt.float32r
    nS = cap // SCH
    nc = bacc.Bacc(None, target_bir_lowering=False)

    xgT = nc.dram_tensor("xgT", [C, cap], f32, kind="ExternalInput")
    w1t = nc.dram_tensor("w1t", [C, F], f32, kind="ExternalInput")
    w2t = nc.dram_tensor("w2t", [F, C], f32, kind="ExternalInput")
    b1r = nc.dram_tensor("b1r", [P, F // P], f32, kind="ExternalInput")
    b2r = nc.dram_tensor("b2r", [P, C], f32, kind="ExternalInput")
    wg = nc.dram_tensor("wg", [P, cap // P], f32, kind="ExternalInput")
    yg = nc.dram_tensor("yg", [cap, C], f32, kind="ExternalOutput")

    w1v = w1t.ap().rearrange("(co ci) f -> ci co f", ci=P)  # [128, 8, F]
    xgv = xgT.ap().rearrange("(co ci) n -> ci co n", ci=P)  # [128, 8, cap]

    with TileContext(nc) as tc:
        with (
            tc.tile_pool(name="consts", bufs=1) as consts,
            tc.tile_pool(name="wpool", bufs=4) as wpool,
            tc.tile_pool(name="xpool", bufs=2) as xpool,
            tc.tile_pool(name="hpool", bufs=3) as hpool,
            tc.tile_pool(name="ypool", bufs=3) as ypool,
            tc.tile_pool(name="psum_h", bufs=2, space="PSUM") as psum_h,
            tc.tile_pool(name="psum_y", bufs=1, space="PSUM") as psum_y,
        ):
            b1_sb = consts.tile([P, F // P], f32)
            nc.sync.dma_start(b1_sb[:], b1r[:, :])
            b2_sb = consts.tile([P, C], f32)
            nc.sync.dma_start(b2_sb[:], b2r[:, :])
            wg_sb = consts.tile([P, cap // P], f32)
            nc.sync.dma_start(wg_sb[:], wg[:, :])

            for s in range(nS):
                xg_s = xpool.tile([P, 8, SCH], f32r, tag="xg")
                nc.sync.dma_start(xg_s[:], xgv[:, :, s * SCH : (s + 1) * SCH].bitcast(f32r))

                yps = [
                    [
                        psum_y.tile(
                            [P, 512], f32, tag=f"y_{t}_{cc}", name=f"y_{t}_{cc}"
                        )
                        for cc in range(2)
                    ]
                    for t in range(3)
                ]

                for f in range(F // P):  # 32
                    w1c = wpool.tile([P, 8, P], f32r, tag="w1c")
                    nc.sync.dma_start(w1c[:], w1v[:, :, f * P : (f + 1) * P].bitcast(f32r))
                    w2c = wpool.tile([P, C], f32r, tag="w2c")
                    nc.sync.dma_start(w2c[:], w2t[f * P : (f + 1) * P, :].bitcast(f32r))

                    hps = psum_h.tile([P, SCH], f32, tag="h")
                    for c in range(8):
                        nc.tensor.matmul(
                            hps[:],
                            lhsT=w1c[:, c, :],
                            rhs=xg_s[:, c, :],
                            start=(c == 0),
                            stop=(c == 7),
                        )
                    hT = hpool.tile([P, SCH], f32r, tag="hT")
                    nc.scalar.activation(
                        hT[:],
                        hps[:],
                        mybir.ActivationFunctionType.Relu,
                        bias=b1_sb[:, f : f + 1],
                        scale=1.0,
                    )
                    for t in range(3):
                        for cc in range(2):
                            nc.tensor.matmul(
                                yps[t][cc][:],
                                lhsT=hT[:, t * P : (t + 1) * P],
                                rhs=w2c[:, cc * 512 : (cc + 1) * 512],
                                start=(f == 0),
                                stop=(f == F // P - 1),
                            )

                for t in range(3):
                    y_sb = ypool.tile([P, C], f32, tag="y_sb")
                    for cc in range(2):
                        sl = slice(cc * 512, (cc + 1) * 512)
                        nc.vector.tensor_add(y_sb[:, sl], yps[t][cc][:], b2_sb[:, sl])
                    yf = ypool.tile([P, C], f32, tag="yf")
                    nc.scalar.mul(yf[:], y_sb[:], wg_sb[:, s * 3 + t : s * 3 + t + 1])
                    nc.sync.dma_start(
                        yg[(s * 3 + t) * P : (s * 3 + t + 1) * P, :], yf[:]
                    )
    nc.compile()
    return nc




def _build_fast(cap: int):
    """Fast path (b1 == 0 and b2 == 0): inputs pre-gated and pre-tiled on host.

    Loop nest: f-groups (NF_G chunks of F) outer, token s-chunks inner.
    Weights stream through SBUF exactly once; per-token-chunk y accumulates
    in SBUF across groups (DVE adds), written out on the last group.
      inputs : xgp [nS, 128, 8, SCH]   gated tokens, tiled for mm1 rhs
               w1p [32, 128, 8, 128]   w1.T tiled for mm1 lhsT
               w2t [4096, 1024]
      output : yg  [cap, 1024]
    """
    import concourse.mybir as mybir
    from concourse import bacc
    from concourse.tile import TileContext

    f32 = mybir.dt.float32
    f32r = mybir.dt.float32r
    nS = cap // SCH
    NF_G = 4  # f-chunks per resident weight group
    NG = (F // P) // NF_G  # 8 groups
    nc = bacc.Bacc(None, target_bir_lowering=False)

    xgp = nc.dram_tensor("xgp", [nS, P, 8, SCH], f32, kind="ExternalInput")
    w1p = nc.dram_tensor("w1p", [F // P, P, 8, P], f32, kind="ExternalInput")
    w2t = nc.dram_tensor("w2t", [F, C], f32, kind="ExternalInput")
    yg = nc.dram_tensor("yg", [cap, C], f32, kind="ExternalOutput")

    with TileContext(nc) as tc:
        with (
            tc.tile_pool(name="ybuf", bufs=1) as ybuf,
            tc.tile_pool(name="wpool", bufs=2) as wpool,
            tc.tile_pool(name="xpool", bufs=2) as xpool,
            tc.tile_pool(name="hpool", bufs=3) as hpool,
            tc.tile_pool(name="psum_h", bufs=2, space="PSUM") as psum_h,
            tc.tile_pool(name="psum_y", bufs=1, space="PSUM") as psum_y,
        ):
            y_all = [
                [
                    ybuf.tile([P, C], f32, name=f"yall_{s}_{t}", tag=f"yall_{s}_{t}")
                    for t in range(3)
                ]
                for s in range(nS)
            ]

            for g in range(NG):
                w1g = wpool.tile([P, NF_G, 8, P], f32r, tag="w1g", name="w1g")
                nc.sync.dma_start(
                    w1g[:],
                    w1p[g * NF_G : (g + 1) * NF_G]
                    .rearrange("f ci co fj -> ci f co fj")
                    .bitcast(f32r),
                )
                w2g = wpool.tile([P, NF_G, C], f32r, tag="w2g", name="w2g")
                nc.sync.dma_start(
                    w2g[:],
                    w2t[g * NF_G * P : (g + 1) * NF_G * P]
                    .rearrange("(f fi) c -> fi f c", fi=P)
                    .bitcast(f32r),
                )

                for s in range(nS):
                    xg_s = xpool.tile([P, 8, SCH], f32r, tag="xg", name="xg_s")
                    nc.sync.dma_start(xg_s[:], xgp[s].bitcast(f32r))

                    yps = [
                        [
                            psum_y.tile(
                                [P, 512], f32, tag=f"y_{t}_{cc}", name=f"y_{t}_{cc}"
                            )
                            for cc in range(2)
                        ]
                        for t in range(3)
                    ]

                    for fl in range(NF_G):
                        hps = psum_h.tile([P, SCH], f32, tag="h", name="hps")
                        for c in range(8):
                            nc.tensor.matmul(
                                hps[:],
                                lhsT=w1g[:, fl, c, :],
                                rhs=xg_s[:, c, :],
                                start=(c == 0),
                                stop=(c == 7),
                            )
                        hT = hpool.tile([P, SCH], f32r, tag="hT", name="hT")
                        nc.scalar.activation(
                            hT[:], hps[:], mybir.ActivationFunctionType.Relu
                        )
                        for t in range(3):
                            for cc in range(2):
                                nc.tensor.matmul(
                                    yps[t][cc][:],
                                    lhsT=hT[:, t * P : (t + 1) * P],
                                    rhs=w2g[:, fl, cc * 512 : (cc + 1) * 512],
                                    start=(fl == 0),
                                    stop=(fl == NF_G - 1),
                                )

                    for t in range(3):
                        ya = y_all[s][t]
                        if g == 0:
                            for cc in range(2):
                                sl = slice(cc * 512, (cc + 1) * 512)
                                nc.vector.tensor_copy(ya[:, sl], yps[t][cc][:])
                        else:
                            for cc in range(2):
                                sl = slice(cc * 512, (cc + 1) * 512)
                                nc.vector.tensor_add(ya[:, sl], ya[:, sl], yps[t][cc][:])
                        if g == NG - 1:
                            nc.sync.dma_start(
                                yg[(s * 3 + t) * P : (s * 3 + t + 1) * P, :], ya[:]
                            )
    nc.compile()
    return nc


_CACHE = {}
_TRACE = False  # test harness sets True to capture an NTFF profile
_LAST_RES = None


def _get_nc(cap, fast):
    key = (cap, fast)
    if key not in _CACHE:
        _CACHE[key] = _build_fast(cap) if fast else _build(cap)
    return _CACHE[key]


def _route(x_flat, router_w):
    """Top-2 routing, float64 for stable selection. Returns idx/weights per expert."""
    logits = x_flat.astype(np.float64) @ router_w.astype(np.float64).T
    t = np.exp(logits - logits.max(-1, keepdims=True))
    p = t / t.sum(-1, keepdims=True)
    top2 = np.argsort(-p, axis=-1)[:, :2]
    pv = np.take_along_axis(p, top2, axis=-1)
    wn = pv / (pv.sum(-1, keepdims=True) + 1e-9)
    return top2, wn


def kernel(x, router_w, w1, b1, w2, b2):
    from concourse.bass_utils import run_bass_kernel_spmd

    Bx, Nx, Cx = x.shape
    x_flat = np.ascontiguousarray(x.reshape(-1, Cx))
    T = x_flat.shape[0]

    top2, wn = _route(x_flat, router_w)
    idxs, gates = [], []
    for e in range(E):
        sel = top2 == e
        we = np.where(sel, wn, 0.0).sum(-1)
        idx = np.nonzero(sel.any(-1))[0]
        idxs.append(idx)
        gates.append(we[idx].astype(np.float32))
    cap = max(len(i) for i in idxs)
    cap = ((cap + SCH - 1) // SCH) * SCH
    nS = cap // SCH

    fast = bool(np.all(b1 == 0) and np.all(b2 == 0))
    nc = _get_nc(cap, fast)

    in_maps = []
    for e in range(E):
        n_e = len(idxs[e])
        xg = np.zeros((cap, Cx), np.float32)
        xg[:n_e] = x_flat[idxs[e]]
        wg = np.zeros(cap, np.float32)
        wg[:n_e] = gates[e]
        if fast:
            xg *= wg[:, None]  # pre-gate: exact since b1 == 0 and wg >= 0
            in_maps.append(
                {
                    "xgp": np.ascontiguousarray(
                        xg.reshape(nS, SCH, 8, P).transpose(0, 3, 2, 1)
                    ),
                    "w1p": np.ascontiguousarray(
                        w1[e].reshape(F // P, P, 8, P).transpose(0, 3, 2, 1)
                    ),
                    "w2t": np.ascontiguousarray(w2[e].T),
                }
            )
        else:
            in_maps.append(
                {
                    "xgT": np.ascontiguousarray(xg.T),
                    "w1t": np.ascontiguousarray(w1[e].T),
                    "w2t": np.ascontiguousarray(w2[e].T),
                    "b1r": np.ascontiguousarray(b1[e].reshape(F // P, P).T),
                    "b2r": np.ascontiguousarray(np.broadcast_to(b2[e], (P, Cx))),
                    "wg": np.ascontiguousarray(wg.reshape(cap // P, P).T),
                }
            )

    global _LAST_RES
    res = run_bass_kernel_spmd(nc, in_maps, core_ids=list(range(E)), trace=_TRACE)
    _LAST_RES = res

    out = np.zeros((T, Cx), np.float32)
    for e in range(E):
        n_e = len(idxs[e])
        out[idxs[e]] += res.results[e]["yg"][:n_e]
    return out.reshape(Bx, Nx, Cx)
